# revision 5
# baseline (speedup 1.0000x reference)
"""Trainium2 Bass kernel for a transformer encoder layer (nn_Encoder).

x:[2,2048,1024] f32. 8 NeuronCores, data-parallel: core c handles batch
n=c//4, query rows qi=c%4 (512 tokens). K/V are recomputed per core for
the full batch (x4 redundancy) to avoid collectives, which are far too
slow (~300us for the 8.4MB all-reduce this would replace).
All matmuls run as float32r (full PE rate, ~1e-4 rel err).
"""
import os
import sys

for _p in ("/opt/trn_rl_repo", "/root/.axon_site/_ro/trn_rl_repo"):
    if os.path.isdir(_p) and _p not in sys.path:
        sys.path.insert(0, _p)

import numpy as np
import concourse.bass as bass
import concourse.mybir as mybir
import concourse.tile as tile
from concourse import bacc
from concourse.bass_utils import run_bass_kernel_spmd
from concourse.masks import make_identity

F32 = mybir.dt.float32
F32R = mybir.dt.float32r
AF = mybir.ActivationFunctionType
ALU = mybir.AluOpType

D = 1024
H = 16
HD = 64
FF = 4096
L = 2048
NB = 2
P = 128
QT = 512          # query tokens per core
DC = D // P       # 8 chunks of the model dim
KT = L // P       # 16 key tiles
FC = FF // P      # 32 ff chunks
TT = QT // P      # 4 own-token tiles
NPAIR = H // 2    # 8 head pairs
EPS = 1e-5

_CACHED_NC = None


def _build_nc():
    nc = bacc.Bacc("TRN2", target_bir_lowering=False)

    din = {}

    def dparam(name, shape, dt=F32R):
        din[name] = nc.dram_tensor(name, shape, dt, kind="ExternalInput")
        return din[name]

    xT = dparam("xT", [D, L])              # x[n].T
    xTq = dparam("xTq", [D, QT])           # own-token columns of xT
    xq = dparam("xq", [QT, D], F32)        # own tokens, natural (residual)
    wq = dparam("wq", [NPAIR, DC, P, P])   # [pair, dc, dpart, cols]
    wk = dparam("wk", [NPAIR, DC, P, P])
    wv = dparam("wv", [2, DC, P, D // 2])  # [half, dc, dpart, 512 vcols]
    wo = dparam("wo", [DC, P, D])          # [hd-chunk, hd-part, ocols]
    w1 = dparam("w1", [FC, DC, P, P])      # [fc, dc, dpart, fcols]
    w2 = dparam("w2", [FC, P, D])          # [fc, ff-part, ocols]
    bq = dparam("bq", [P, NPAIR], F32)     # per-partition bias, by pair
    bk = dparam("bk", [P, NPAIR], F32)
    b1 = dparam("b1", [P, FC], F32)
    bvb = dparam("bvb", [P, D], F32)       # host-broadcast per-column params
    bob = dparam("bob", [P, D], F32)
    b2b = dparam("b2b", [P, D], F32)
    g1b = dparam("g1b", [P, D], F32)
    be1b = dparam("be1b", [P, D], F32)
    g2b = dparam("g2b", [P, D], F32)
    be2b = dparam("be2b", [P, D], F32)
    vones = dparam("vones", [P, KT], F32R)

    y = nc.dram_tensor("y", [QT, D], F32, kind="ExternalOutput")
    kT_dram = nc.dram_tensor("kT_scratch", [NPAIR, P, L], F32R)

    with tile.TileContext(nc) as tc:
        with tc.tile_pool(name="pers", bufs=1) as pers:
            ident = pers.tile([P, P], F32, tag="ident")
            make_identity(nc, ident[:])
            bq_t = pers.tile([P, NPAIR], F32, tag="bq")
            bk_t = pers.tile([P, NPAIR], F32, tag="bk")
            b1_t = pers.tile([P, FC], F32, tag="b1")
            eps_t = pers.tile([P, 1], F32, tag="eps")
            nc.gpsimd.memset(eps_t[:], EPS)
            nc.sync.dma_start(bq_t[:], bq[:])
            nc.sync.dma_start(bk_t[:], bk[:])
            nc.sync.dma_start(b1_t[:], b1[:])

            # ---- persistent blobs (tag-shared slots across phases) ----
            # blobA: xT (proj) -> ff1T (ffn);  both 64KB/partition
            # blobB: v_aug (proj+attn) -> wo (out-proj) -> hT (ffn)
            # tok1:  xTq (q-proj) -> outSB (attn out, transposed)
            # tok2:  qT (proj+attn) -> h (post-LN1, natural)
            xT_t = pers.tile([P, DC, L], F32R, tag="blobA")
            v_aug = pers.tile([P, KT, H * (HD + 1)], F32R, tag="blobB")
            ones_t = pers.tile([P, KT], F32R, tag="ones")
            nc.sync.dma_start(ones_t[:], vones[:])
            nc.vector.tensor_copy(
                v_aug[:].rearrange("p t (h c) -> p t h c", c=HD + 1)[:, :, :, HD],
                ones_t[:, :, None].to_broadcast([P, KT, H]))

            # ================= K projection (to DRAM scratch) =========
            with tc.tile_pool(name="kp", bufs=2) as kp, \
                 tc.tile_pool(name="kps", bufs=4, space="PSUM") as kps:
                nc.sync.dma_start(xT_t[:], xT.rearrange("(c p) t -> p c t", p=P))
                for pr in range(NPAIR):
                    wk_t = kp.tile([P, DC, P], F32R, tag="w")
                    nc.sync.dma_start(wk_t[:], wk[pr].rearrange("c p m -> p c m"))
                    kT_sb = kp.tile([P, L], F32R, tag="kts")
                    for t4 in range(4):
                        ps = kps.tile([P, 512], F32, tag="mm")
                        for dc in range(DC):
                            nc.tensor.matmul(
                                ps[:], wk_t[:, dc, :],
                                xT_t[:, dc, t4 * 512:(t4 + 1) * 512],
                                start=(dc == 0), stop=(dc == DC - 1))
                        nc.vector.tensor_scalar(
                            kT_sb[:, t4 * 512:(t4 + 1) * 512], ps[:],
                            bk_t[:, pr:pr + 1], None, ALU.add)
                    nc.sync.dma_start(kT_dram[pr], kT_sb[:])

            # ================= V projection ===========================
            with tc.tile_pool(name="vp", bufs=1) as vp, \
                 tc.tile_pool(name="vps", bufs=4, space="PSUM") as vps:
                bvb_t = vp.tile([P, D], F32, tag="bvb")
                nc.sync.dma_start(bvb_t[:], bvb[:])
                for half in range(2):
                    wv_t = vp.tile([P, DC, 512], F32R, tag="wvh")
                    nc.sync.dma_start(wv_t[:], wv[half].rearrange("c p m -> p c m"))
                    for tt in range(KT):
                        ps = vps.tile([P, 512], F32, tag="mm")
                        for dc in range(DC):
                            nc.tensor.matmul(
                                ps[:], xT_t[:, dc, tt * P:(tt + 1) * P],
                                wv_t[:, dc, :],
                                start=(dc == 0), stop=(dc == DC - 1))
                        # strided drain into v_aug (65-wide per head, col 64
                        # stays 1.0 from the memset -> fused softmax denom)
                        dst = v_aug[:, tt, :].rearrange(
                            "p (h c) -> p h c", c=HD + 1)[:, half * 8:(half + 1) * 8, 0:HD]
                        nc.vector.tensor_tensor(
                            dst, ps[:].rearrange("p (h c) -> p h c", c=HD),
                            bvb_t[:, half * 512:(half + 1) * 512].rearrange(
                                "p (h c) -> p h c", c=HD),
                            ALU.add)

            # ================= Q projection (own tokens) ==============
            qT_t = pers.tile([P, NPAIR, QT], F32R, tag="tok2")
            with tc.tile_pool(name="qp", bufs=2) as qp, \
                 tc.tile_pool(name="qps", bufs=4, space="PSUM") as qps:
                xTq_t = pers.tile([P, DC, QT], F32R, tag="tok1")
                nc.sync.dma_start(xTq_t[:], xTq.rearrange("(c p) t -> p c t", p=P))
                for pr in range(NPAIR):
                    wq_t = qp.tile([P, DC, P], F32R, tag="w")
                    nc.sync.dma_start(wq_t[:], wq[pr].rearrange("c p m -> p c m"))
                    ps = qps.tile([P, 512], F32, tag="mm")
                    for dc in range(DC):
                        nc.tensor.matmul(ps[:], wq_t[:, dc, :], xTq_t[:, dc, :],
                                         start=(dc == 0), stop=(dc == DC - 1))
                    nc.vector.tensor_scalar(qT_t[:, pr, :], ps[:],
                                            bq_t[:, pr:pr + 1], None, ALU.add)

            # ================= Attention ==============================
            outSB = pers.tile([P, NPAIR, QT], F32R, tag="tok1")
            groups = [(0, 3), (3, 6), (6, 9), (9, 12), (12, 15), (15, 16)]
            with tc.tile_pool(name="atk", bufs=2) as atk, \
                 tc.tile_pool(name="atd", bufs=2) as atd, \
                 tc.tile_pool(name="stp", bufs=2, space="PSUM") as stpool, \
                 tc.tile_pool(name="pvp", bufs=2, space="PSUM") as pvpool:
                for pr in range(NPAIR):
                    ktp = atk.tile([P, L], F32R, tag="ktp")
                    nc.sync.dma_start(ktp[:], kT_dram[pr])
                    for h2 in range(2):
                        h_idx = 2 * pr + h2
                        rows = slice(h2 * HD, h2 * HD + HD)
                        pvp = pvpool.tile([P, QT], F32, tag="pv")
                        for (a, b) in groups:
                            g = b - a
                            stp = stpool.tile([P, 1536], F32, tag="st")
                            for j in range(g):
                                kt = a + j
                                nc.tensor.matmul(
                                    stp[:, j * 512:(j + 1) * 512],
                                    ktp[rows, kt * P:(kt + 1) * P],
                                    qT_t[rows, pr, :], start=True, stop=True)
                            ptt = atk.tile([P, 3, QT], F32R, tag="pt")
                            nc.scalar.activation(
                                ptt[:, :g, :],
                                stp[:, :g * 512].rearrange("p (c n) -> p c n", n=512),
                                AF.Exp, scale=0.125)
                            for j in range(g):
                                kt = a + j
                                vsl = v_aug[:, kt, :].rearrange(
                                    "p (h c) -> p h c", c=HD + 1)[:, h_idx, :]
                                nc.tensor.matmul(pvp[:HD + 1, :], vsl, ptt[:, j, :],
                                                 start=(kt == 0), stop=(kt == KT - 1))
                        den = atd.tile([1, QT], F32, tag="den")
                        nc.vector.reciprocal(den[:], pvp[HD:HD + 1, :])
                        denb = atd.tile([HD, QT], F32, tag="denb")
                        nc.gpsimd.partition_broadcast(denb[:], den[:])
                        nc.vector.tensor_tensor(outSB[rows, pr, :], pvp[:HD, :],
                                                denb[:], ALU.mult)

            # ================= Output proj + residual + LN1 ===========
            h_t = pers.tile([P, TT, D], F32, tag="tok2")
            with tc.tile_pool(name="op", bufs=2) as op, \
                 tc.tile_pool(name="lnw", bufs=1) as lnw, \
                 tc.tile_pool(name="lnp3", bufs=1) as lnp3, \
                 tc.tile_pool(name="ops", bufs=4, space="PSUM") as ops:
                wo_t = pers.tile([P, DC, D], F32R, tag="blobB")
                nc.sync.dma_start(wo_t[:], wo.rearrange("c p m -> p c m"))
                bob_t = lnw.tile([P, D], F32, tag="bob")
                g1b_t = lnw.tile([P, D], F32, tag="g1b")
                be1b_t = lnw.tile([P, D], F32, tag="be1b")
                nc.sync.dma_start(bob_t[:], bob[:])
                nc.sync.dma_start(g1b_t[:], g1b[:])
                nc.sync.dma_start(be1b_t[:], be1b[:])
                for tt in range(TT):
                    xq_s = op.tile([P, D], F32, tag="xqs")
                    nc.sync.dma_start(
                        xq_s[:], xq.rearrange("(t p) d -> p t d", p=P)[:, tt, :])
                    for oc in range(2):
                        ps = ops.tile([P, 512], F32, tag="mm")
                        for pr in range(NPAIR):
                            nc.tensor.matmul(
                                ps[:], outSB[:, pr, tt * P:(tt + 1) * P],
                                wo_t[:, pr, oc * 512:(oc + 1) * 512],
                                start=(pr == 0), stop=(pr == NPAIR - 1))
                        nc.vector.tensor_tensor(
                            h_t[:, tt, oc * 512:(oc + 1) * 512], ps[:],
                            xq_s[:, oc * 512:(oc + 1) * 512], ALU.add)
                    nc.vector.tensor_tensor(h_t[:, tt, :], h_t[:, tt, :],
                                            bob_t[:], ALU.add)
                    _layernorm(nc, lnp3, h_t[:, tt, :], h_t[:, tt, :],
                               g1b_t[:], be1b_t[:], eps_t)

            # ================= FFN + LN2 ==============================
            with tc.tile_pool(name="fp", bufs=2) as fp, \
                 tc.tile_pool(name="ft", bufs=2) as ft, \
                 tc.tile_pool(name="lnp4", bufs=1) as lnp4, \
                 tc.tile_pool(name="fw", bufs=1) as fw:
                hT_t = pers.tile([P, DC, QT], F32R, tag="blobB")
                with tc.tile_pool(name="tps", bufs=2, space="PSUM") as tps:
                    for tt in range(TT):
                        for dc in range(DC):
                            pst = tps.tile([P, P], F32, tag="tp")
                            nc.tensor.transpose(
                                pst[:], h_t[:, tt, dc * P:(dc + 1) * P], ident[:])
                            nc.vector.tensor_copy(
                                hT_t[:, dc, tt * P:(tt + 1) * P], pst[:])

                ff1 = pers.tile([P, FC, QT], F32R, tag="blobA")
                with tc.tile_pool(name="f1s", bufs=4, space="PSUM") as f1s:
                    for fc in range(FC):
                        w1_t = fp.tile([P, DC, P], F32R, tag="w1")
                        nc.sync.dma_start(w1_t[:], w1[fc].rearrange("c p m -> p c m"))
                        ps = f1s.tile([P, 512], F32, tag="mm")
                        for dc in range(DC):
                            nc.tensor.matmul(ps[:], w1_t[:, dc, :], hT_t[:, dc, :],
                                             start=(dc == 0), stop=(dc == DC - 1))
                        # fused bias + relu
                        nc.vector.tensor_scalar(ff1[:, fc, :], ps[:],
                                                b1_t[:, fc:fc + 1], 0.0,
                                                ALU.add, ALU.max)

                b2b_t = fw.tile([P, D], F32, tag="b2b")
                g2b_t = fw.tile([P, D], F32, tag="g2b")
                be2b_t = fw.tile([P, D], F32, tag="be2b")
                nc.sync.dma_start(b2b_t[:], b2b[:])
                nc.sync.dma_start(g2b_t[:], g2b[:])
                nc.sync.dma_start(be2b_t[:], be2b[:])
                with tc.tile_pool(name="f2s", bufs=1, space="PSUM") as f2s:
                    pss = [f2s.tile([P, 512], F32, tag=f"ff2_{i}", name=f"ff2_{i}") for i in range(8)]
                    for fc in range(FC):
                        w2_t = fp.tile([P, D], F32R, tag="w2")
                        nc.sync.dma_start(w2_t[:], w2[fc])
                        for tt in range(TT):
                            for oc in range(2):
                                nc.tensor.matmul(
                                    pss[tt * 2 + oc],
                                    ff1[:, fc, tt * P:(tt + 1) * P],
                                    w2_t[:, oc * 512:(oc + 1) * 512],
                                    start=(fc == 0), stop=(fc == FC - 1))
                    for tt in range(TT):
                        t2 = ft.tile([P, D], F32, tag="t2")
                        for oc in range(2):
                            nc.vector.tensor_tensor(
                                t2[:, oc * 512:(oc + 1) * 512],
                                pss[tt * 2 + oc],
                                h_t[:, tt, oc * 512:(oc + 1) * 512], ALU.add)
                        nc.vector.tensor_tensor(t2[:], t2[:], b2b_t[:], ALU.add)
                        _layernorm(nc, lnp4, t2[:], t2[:], g2b_t[:], be2b_t[:], eps_t)
                        nc.sync.dma_start(
                            y.rearrange("(t p) d -> p t d", p=P)[:, tt, :], t2[:])

    nc.compile()
    return nc


def _layernorm(nc, pool, dst, src, g_t, be_t, eps_t):
    """dst = (src - mean)/sqrt(var + eps) * g + be, row-wise over 1024."""
    mu = pool.tile([P, 1], F32, tag="ln_mu")
    nc.vector.tensor_reduce(mu[:], src, mybir.AxisListType.X, ALU.add)
    nc.vector.tensor_scalar_mul(mu[:], mu[:], 1.0 / D)
    c = pool.tile([P, D], F32, tag="ln_c")
    nc.vector.tensor_scalar(c[:], src, mu[:], None, ALU.subtract)
    sq = pool.tile([P, D], F32, tag="ln_sq")
    nc.vector.tensor_tensor(sq[:], c[:], c[:], ALU.mult)
    ss = pool.tile([P, 1], F32, tag="ln_ss")
    nc.vector.tensor_reduce(ss[:], sq[:], mybir.AxisListType.X, ALU.add)
    s = pool.tile([P, 1], F32, tag="ln_s")
    nc.scalar.activation(s[:], ss[:], AF.Sqrt, bias=eps_t[:], scale=1.0 / D)
    r = pool.tile([P, 1], F32, tag="ln_r")
    nc.vector.reciprocal(r[:], s[:])
    # one Newton step: r <- r * (1.5 - 0.5 * v * r^2), v = ss/D + eps
    v = pool.tile([P, 1], F32, tag="ln_v")
    nc.vector.tensor_scalar(v[:], ss[:], 1.0 / D, EPS, ALU.mult, ALU.add)
    t = pool.tile([P, 1], F32, tag="ln_t")
    nc.vector.tensor_tensor(t[:], r[:], r[:], ALU.mult)
    nc.vector.tensor_tensor(t[:], t[:], v[:], ALU.mult)
    nc.vector.tensor_scalar(t[:], t[:], -0.5, 1.5, ALU.mult, ALU.add)
    nc.vector.tensor_tensor(r[:], r[:], t[:], ALU.mult)
    nc.vector.tensor_scalar(dst, c[:], r[:], None, ALU.mult)
    nc.vector.tensor_tensor(dst, dst, g_t, ALU.mult)
    nc.vector.tensor_tensor(dst, dst, be_t, ALU.add)


def make_in_maps(x, w_qkv, b_qkv, w_o, b_o, g1, be1, w1, b1, w2, b2, g2, be2):
    f = np.float32
    x = np.asarray(x, f)
    w_qkv = np.asarray(w_qkv, f)
    b_qkv = np.asarray(b_qkv, f)
    bc = lambda v: np.ascontiguousarray(
        np.broadcast_to(np.asarray(v, f).reshape(1, D), (P, D)))
    shared = {
        "wq": np.ascontiguousarray(
            w_qkv[:, :D].reshape(DC, P, NPAIR, P).transpose(2, 0, 1, 3)),
        "wk": np.ascontiguousarray(
            w_qkv[:, D:2 * D].reshape(DC, P, NPAIR, P).transpose(2, 0, 1, 3)),
        "wv": np.ascontiguousarray(
            w_qkv[:, 2 * D:].reshape(DC, P, 2, 512).transpose(2, 0, 1, 3)),
        "wo": np.ascontiguousarray(np.asarray(w_o, f).reshape(DC, P, D)),
        "w1": np.ascontiguousarray(
            np.asarray(w1, f).reshape(DC, P, FC, P).transpose(2, 0, 1, 3)),
        "w2": np.ascontiguousarray(np.asarray(w2, f).reshape(FC, P, D)),
        "bq": np.ascontiguousarray(b_qkv[:D].reshape(NPAIR, P).T),
        "bk": np.ascontiguousarray(b_qkv[D:2 * D].reshape(NPAIR, P).T),
        "b1": np.ascontiguousarray(np.asarray(b1, f).reshape(FC, P).T),
        "bvb": bc(b_qkv[2 * D:]), "bob": bc(b_o), "b2b": bc(b2),
        "g1b": bc(g1), "be1b": bc(be1), "g2b": bc(g2), "be2b": bc(be2),
        "vones": np.ones((P, KT), f),
    }
    in_maps = []
    for c in range(8):
        n, qi = divmod(c, 4)
        xTn = np.ascontiguousarray(x[n].T)
        m = dict(shared)
        m["xT"] = xTn
        m["xTq"] = np.ascontiguousarray(xTn[:, qi * QT:(qi + 1) * QT])
        m["xq"] = np.ascontiguousarray(x[n, qi * QT:(qi + 1) * QT, :])
        in_maps.append(m)
    return in_maps


def get_nc():
    global _CACHED_NC
    if _CACHED_NC is None:
        _CACHED_NC = _build_nc()
    return _CACHED_NC


def kernel(**inputs):
    in_maps = make_in_maps(**inputs)
    nc = get_nc()
    res = run_bass_kernel_spmd(nc, in_maps, list(range(8))).results
    y = np.empty((NB, L, D), np.float32)
    for c in range(8):
        n, qi = divmod(c, 4)
        y[n, qi * QT:(qi + 1) * QT] = res[c]["y"]
    return y


if __name__ == "__main__":
    rng = np.random.default_rng(0)
    demo = {
        "x": rng.standard_normal((NB, L, D)).astype(np.float32),
        "w_qkv": rng.standard_normal((D, 3 * D)).astype(np.float32) * 0.03,
        "b_qkv": rng.standard_normal(3 * D).astype(np.float32) * 0.03,
        "w_o": rng.standard_normal((D, D)).astype(np.float32) * 0.03,
        "b_o": rng.standard_normal(D).astype(np.float32) * 0.03,
        "g1": np.ones(D, np.float32), "be1": np.zeros(D, np.float32),
        "w1": rng.standard_normal((D, FF)).astype(np.float32) * 0.03,
        "b1": rng.standard_normal(FF).astype(np.float32) * 0.03,
        "w2": rng.standard_normal((FF, D)).astype(np.float32) * 0.015,
        "b2": rng.standard_normal(D).astype(np.float32) * 0.015,
        "g2": np.ones(D, np.float32), "be2": np.zeros(D, np.float32),
    }
    out = kernel(**demo)
    print("kernel output:", out.shape, out.dtype, np.abs(out).mean())
